# revision 6
# baseline (speedup 1.0000x reference)
"""Trainium2 Bass kernel v2: 8-head causal MHA with RoPE, B=2 T=2048 E=1024 H=8 D=512.

Sharding: 8 cores = 2 (batch) x 4 (head-pair) tensor-parallel groups.
Each core computes q/k/v projections for its 2 heads, causal attention,
and a row-parallel partial of the output projection; the host sums the
4 partials per batch (unshard) and transposes back to [B, T, E].

v2 changes vs baseline:
- All SBUF matmul operands in bf16 (same 1 cyc/row PE rate as f32r,
  half the DMA bytes and SBUF footprint). Output written bf16 and
  upcast host-side.
- Both heads' q/k/v and attention outputs stay resident in SBUF --
  no DRAM spill/reload of head-0's attention output.
- x is DMA'd in four 512-column blocks so the first projection
  matmuls start after ~5us instead of waiting for the full x load.
- Attention blocks of the two heads are interleaved (h0 ib, h1 ib,
  h0 ib+1, ...) and the softmax normalize chain of each block is
  deferred and emitted after the next block's first score matmuls,
  so the PE never sits behind the reciprocal/broadcast chain.
- The denominator reciprocal is broadcast across partitions with a
  [1,128]x[1,256] PE matmul instead of the slow gpsimd
  partition_broadcast.
- scores(jt+1) is emitted before attnV(jt) (software pipelining) so
  the PE is never waiting on the exp activation latency.
- wo output-projection blocks are emitted inside the attention
  stream (one 512-col block after every other ib pair), overlapping
  the final projection with attention instead of serializing it.
"""

import os
import sys

for _p in ("/opt/trn_rl_repo", "/root/.axon_site/_ro/trn_rl_repo"):
    if os.path.isdir(_p) and _p not in sys.path:
        sys.path.insert(0, _p)

import numpy as np
import ml_dtypes

import concourse.bacc as bacc
import concourse.bass_isa as bass_isa
import concourse.mybir as mybir
import concourse.tile as tile
from concourse.bass_utils import run_bass_kernel_spmd

B, T, E, H, D = 2, 2048, 1024, 8, 512
P = 128
NE = E // P          # 8 e-chunks (contraction)
NTB = T // 512       # 4 t-blocks of 512
NTT = T // P         # 16 t-tiles of 128
NDC = D // P         # 4 d-chunks per head
HPC = 2              # heads per core
G = 4                # head groups (cores per batch)
IBW = 256            # attention query-block width
NIB = T // IBW       # 8 query blocks

BF16 = mybir.dt.bfloat16
F32R = mybir.dt.float32r
F32 = mybir.dt.float32
F16 = mybir.dt.float16

MASK_NEG = -30000.0

# repeat the whole body R times inside the NEFF (device-time measurement via
# wall-clock amplification; always 1 for real use)
KREPS = int(os.environ.get("KREPS", "1"))


def _build_nc():
    nc = bacc.Bacc("TRN2", target_bir_lowering=False, debug=False, num_devices=8)

    xT = nc.declare_dram_parameter("xT", [E, T], BF16, isOutput=False)
    wqT = nc.declare_dram_parameter("wqT", [E, HPC * D], BF16, isOutput=False)
    wkT = nc.declare_dram_parameter("wkT", [E, HPC * D], BF16, isOutput=False)
    wvT = nc.declare_dram_parameter("wvT", [E, HPC * D], BF16, isOutput=False)
    woT = nc.declare_dram_parameter("woT", [HPC * D, E], BF16, isOutput=False)
    cosT = nc.declare_dram_parameter("cosT", [D // 2, T], F16, isOutput=False)
    sinT = nc.declare_dram_parameter("sinT", [D // 2, T], F16, isOutput=False)
    masks = nc.declare_dram_parameter("masks", [2 * P, IBW], F16, isOutput=False)

    outT = nc.declare_dram_parameter("outT", [E, T], BF16, isOutput=True)

    Copy = mybir.ActivationFunctionType.Copy
    Exp = mybir.ActivationFunctionType.Exp

    with tile.TileContext(nc) as tc:
        with (
            tc.tile_pool(name="glob", bufs=1) as gp,
            tc.tile_pool(name="right", bufs=1, side="right") as rp,
            tc.tile_pool(name="psum", bufs=1, space="PSUM") as pp,
        ):
            mask_t = rp.tile([P, 2, IBW], F16, tag="masks")

            def blk(name, dtype=BF16, w=512):
                return gp.tile([P, w], dtype, tag="blk", bufs=6, name=name)

            for rep in range(KREPS):
                qr, kr, vv, oT = {}, {}, {}, {}
                for h in range(HPC):
                    qr[h] = rp.tile([P, NDC, T], BF16, tag=f"qr{h}", name=f"qr{h}_{rep}")
                    kr[h] = rp.tile([P, NDC, T], BF16, tag=f"kr{h}", name=f"kr{h}_{rep}")
                    vv[h] = rp.tile([P, NTT, D], BF16, tag=f"vv{h}", name=f"vv{h}_{rep}")
                    oT[h] = rp.tile([P, NDC, T], BF16, tag=f"oT{h}", name=f"oT{h}_{rep}")

                # deferred-emission queue: wo blocks are emitted after the
                # NEXT attention block's first score matmuls so their PSUM
                # reuse and oT reads never head-block the PE queue.
                pending = []

                def flush():
                    while pending:
                        pending.pop(0)()

                # cross-block attnV carry: each attention block's last two
                # attnV groups are emitted inside the NEXT block (or the next
                # projection), keeping a global 2-score-group lag so the PE
                # never waits on the exp chain even in shallow blocks
                carry = []

                def drain():
                    while carry:
                        carry.pop(0)()

                def emit_attn(h, ib):
                    icols = slice(ib * IBW, (ib + 1) * IBW)
                    jt_max = 2 * ib + 1
                    po = [pp.tile([P, 512], F32, tag="a", bufs=4,
                                  name=f"po{h}{ib}{half}_{rep}") for half in range(2)]
                    # per-partition partial softmax denominators, accumulated
                    # on DVE as each exp tile lands (keeps the PE out of it)
                    acc = gp.tile([P, IBW], F32, tag="acc", bufs=2,
                                  name=f"acc{h}{ib}_{rep}")
                    e_ts = {}

                    def scores(jt):
                        # the outermost diagonal tile (jt == jt_max) is fully
                        # masked for the first half of the query block: compute
                        # only query sub-cols [128:256] there
                        off = P if jt == jt_max else 0
                        w = IBW - off
                        qcols = slice(ib * IBW + off, (ib + 1) * IBW)
                        ps = pp.tile([P, w], F32, tag="b", bufs=4,
                                     name=f"ps{h}{ib}{jt}_{rep}")
                        for dc in range(NDC):
                            nc.tensor.matmul(ps[:],
                                             kr[h][:, dc, jt * P : (jt + 1) * P],
                                             qr[h][:, dc, qcols],
                                             start=(dc == 0), stop=(dc == NDC - 1))
                        if jt >= 2 * ib:
                            # both diagonal tiles reduce to the same triangle
                            nc.vector.tensor_add(ps[:], ps[:], mask_t[:, 0, 0:w])
                        e_t = gp.tile([P, w], BF16, tag="e", bufs=5,
                                      name=f"et{h}{ib}{jt}_{rep}")
                        nc.scalar.activation(e_t[:], ps[:], Exp)
                        if jt == 0:
                            nc.vector.tensor_copy(acc[:], e_t[:])
                        elif jt == 1:
                            # query cols [0:128] only ever see jt 0: finalize
                            # them by the copy; jt>=1 tiles add full width
                            nc.vector.tensor_add(acc[:, off:IBW],
                                                 acc[:, off:IBW], e_t[:])
                        else:
                            nc.vector.tensor_add(acc[:, off:IBW],
                                                 acc[:, off:IBW], e_t[:])
                        e_ts[jt] = e_t

                    def attnv(jt):
                        e_t = e_ts.pop(jt)
                        off = P if jt == jt_max else 0
                        for dc in range(NDC):
                            # one accumulation group per po bank: start
                            # zeroes the whole bank, so only the first
                            # matmul starts and only the last stops
                            nc.tensor.matmul(
                                po[dc // 2][:, (dc % 2) * IBW + off : (dc % 2 + 1) * IBW],
                                vv[h][:, jt, dc * P : (dc + 1) * P], e_t[:],
                                start=(jt == 0 and dc % 2 == 0),
                                stop=(jt == jt_max and dc % 2 == 1),
                                skip_group_check=True)

                    def normalize():
                        # no PE ops in this chain: gpsimd all-reduce over the
                        # k partitions lands the denominator on every
                        # partition at once, DVE reciprocal + muls finish it
                        den = gp.tile([P, IBW], F32, tag="den", bufs=2,
                                      name=f"den{h}{ib}_{rep}")
                        nc.gpsimd.partition_all_reduce(den[:], acc[:], channels=P,
                                                       reduce_op=bass_isa.ReduceOp.add)
                        rcp = gp.tile([P, IBW], F32, tag="rcpw", bufs=2,
                                      name=f"rcp{h}{ib}_{rep}")
                        nc.vector.reciprocal_approx_fast(rcp[:], den[:])
                        for dc in range(NDC):
                            nc.vector.tensor_mul(
                                oT[h][:, dc, icols],
                                po[dc // 2][:, (dc % 2) * IBW : (dc % 2 + 1) * IBW],
                                rcp[:])

                    # global software pipeline: attnv(jt) runs two score
                    # groups behind scores(jt); the last two attnv groups
                    # ride the carry into the next block / projection
                    scores(0)
                    scores(1)
                    drain()
                    flush()
                    for jt in range(2, jt_max + 1):
                        scores(jt)
                        attnv(jt - 2)
                    carry.append(lambda: attnv(jt_max - 1))
                    carry.append(lambda: (attnv(jt_max), normalize()))

                with tc.tile_pool(name="left", bufs=1) as lp:
                    def load_ws(wdram, h, dp, nm, halves=False):
                        ws = lp.tile([P, NE, 256], BF16, tag="ws", bufs=2, name=nm)
                        wr = wdram.rearrange("(c p) d -> p c d", p=P)
                        lo = h * D + dp * 256
                        if halves:
                            nc.sync.dma_start(ws[:, :, 0:128], wr[:, :, lo : lo + 128])
                            nc.sync.dma_start(ws[:, :, 128:256],
                                              wr[:, :, lo + 128 : lo + 256])
                        else:
                            nc.sync.dma_start(ws[:], wr[:, :, lo : lo + 256])
                        return ws

                    xt = lp.tile([P, NE, T], BF16, tag="xt")
                    cos_t = lp.tile([P, 2, T], F16, tag="cos")
                    sin_t = lp.tile([P, 2, T], F16, tag="sin")

                    # DMA order tuned so the first projection group can start
                    # after ws_q00 + x block 0 (~5us) while the rest streams in.
                    ws_q00 = load_ws(wqT, 0, 0, "ws_q00")
                    xr = xT.rearrange("(c p) t -> p c t", p=P)
                    cosr = cosT.rearrange("(d p) t -> p d t", p=P)
                    sinr = sinT.rearrange("(d p) t -> p d t", p=P)
                    # x blocks and trig halves ordered so neither the matmuls
                    # nor rope ever wait; tiny tensors (ones/masks) go last
                    nc.sync.dma_start(xt[:, :, 0:256], xr[:, :, 0:256])
                    nc.sync.dma_start(xt[:, :, 256:512], xr[:, :, 256:512])
                    nc.sync.dma_start(xt[:, :, 512:1024], xr[:, :, 512:1024])
                    nc.sync.dma_start(cos_t[:, :, 0:512], cosr[:, :, 0:512])
                    nc.sync.dma_start(sin_t[:, :, 0:512], sinr[:, :, 0:512])
                    nc.sync.dma_start(xt[:, :, 1024:1536], xr[:, :, 1024:1536])
                    nc.sync.dma_start(cos_t[:, :, 512:1024], cosr[:, :, 512:1024])
                    nc.sync.dma_start(sin_t[:, :, 512:1024], sinr[:, :, 512:1024])
                    nc.sync.dma_start(xt[:, :, 1536:2048], xr[:, :, 1536:2048])
                    nc.sync.dma_start(cos_t[:, :, 1024:2048], cosr[:, :, 1024:2048])
                    nc.sync.dma_start(sin_t[:, :, 1024:2048], sinr[:, :, 1024:2048])
                    if rep == 0:
                        nc.sync.dma_start(mask_t[:],
                                          masks.rearrange("(q p) c -> p q c", p=P))

                    def proj_qk(h, wdram, dst, tname, ws0=None,
                                drain_after_first=False, tbw=512):
                        """q/k projection with fused rope into dst [P, NDC, T] bf16."""
                        for dp in range(2):
                            if ws0 is not None and dp == 0:
                                ws = ws0
                            else:
                                ws = load_ws(wdram, h, dp, f"ws_{tname}{h}{dp}")
                            for tb in range(T // tbw):
                                if drain_after_first and (dp, tb) == (0, 1):
                                    drain()
                                cols = slice(tb * tbw, (tb + 1) * tbw)
                                psA = pp.tile([P, tbw], F32, tag="a", bufs=4,
                                              name=f"psA_{tname}{h}{dp}{tb}_{rep}")
                                psB = pp.tile([P, tbw], F32, tag="b", bufs=4,
                                              name=f"psB_{tname}{h}{dp}{tb}_{rep}")
                                for c in range(NE):
                                    nc.tensor.matmul(psA[:], ws[:, c, 0:128],
                                                     xt[:, c, cols],
                                                     start=(c == 0), stop=(c == NE - 1))
                                for c in range(NE):
                                    nc.tensor.matmul(psB[:], ws[:, c, 128:256],
                                                     xt[:, c, cols],
                                                     start=(c == 0), stop=(c == NE - 1))
                                ct = cos_t[:, dp, cols]
                                st = sin_t[:, dp, cols]
                                t_ac = blk(f"tac{h}{dp}{tb}{tname}", w=tbw)
                                t_bs = blk(f"tbs{h}{dp}{tb}{tname}", w=tbw)
                                nc.vector.tensor_mul(t_ac[:], psA[:], ct)
                                nc.vector.tensor_mul(t_bs[:], psB[:], st)
                                nc.vector.tensor_sub(dst[:, dp, cols], t_ac[:], t_bs[:])
                                t_as = blk(f"tas{h}{dp}{tb}{tname}", w=tbw)
                                t_bc = blk(f"tbc{h}{dp}{tb}{tname}", w=tbw)
                                nc.vector.tensor_mul(t_as[:], psA[:], st)
                                nc.vector.tensor_mul(t_bc[:], psB[:], ct)
                                nc.vector.tensor_add(dst[:, dp + 2, cols], t_as[:], t_bc[:])

                    def proj_v(h):
                        for dh in range(2):
                            ws = load_ws(wvT, h, dh, f"ws_v{h}{dh}")
                            for tt in range(NTT):
                                psV = pp.tile([P, 256], F32, tag="b", bufs=4,
                                              name=f"psV{h}{dh}{tt}_{rep}")
                                for c in range(NE):
                                    nc.tensor.matmul(psV[:], xt[:, c, tt * P : (tt + 1) * P],
                                                     ws[:, c, :],
                                                     start=(c == 0), stop=(c == NE - 1))
                                nc.scalar.activation(vv[h][:, tt, dh * 256 : (dh + 1) * 256],
                                                     psV[:], Copy)

                    proj_qk(0, wqT, qr[0], "q", ws0=ws_q00, tbw=256)
                    proj_qk(0, wkT, kr[0], "k")
                    proj_v(0)
                    emit_attn(0, 0)
                    emit_attn(0, 1)
                    proj_qk(1, wqT, qr[1], "q", drain_after_first=True)
                    proj_qk(1, wkT, kr[1], "k")
                    proj_v(1)

                # left pool released: x/trig/ws space becomes the wo-phase pool
                with tc.tile_pool(name="left2", bufs=1) as lp2:
                    wo_t = lp2.tile([P, NE, E], BF16, tag="wo_t")
                    wor = woT.rearrange("(c p) e -> p c e", p=P)
                    for c in range(NE):
                        nc.sync.dma_start(wo_t[:, c, :], wor[:, c, :])

                    def wo_block(tb, w=512, pair=True):
                        def emit():
                            for sub in range(512 // w):
                                cols = slice(tb * 512 + sub * w,
                                             tb * 512 + (sub + 1) * w)
                                pws = {}
                                stages = {}

                                def first_half(et):
                                    pw = pp.tile([P, w], F32, tag="b", bufs=4,
                                                 name=f"pw{tb}{et}{sub}_{rep}")
                                    pws[et] = pw
                                    for hc in range(NDC):
                                        nc.tensor.matmul(
                                            pw[:], wo_t[:, hc, et * P : (et + 1) * P],
                                            oT[0][:, hc, cols],
                                            start=(hc == 0), stop=False)

                                def second_half(et):
                                    pw = pws.pop(et)
                                    for hc in range(NDC, NE):
                                        nc.tensor.matmul(
                                            pw[:], wo_t[:, hc, et * P : (et + 1) * P],
                                            oT[1][:, hc - NDC, cols],
                                            start=False, stop=(hc == NE - 1))
                                    if not pair:
                                        # tail block: per-et DMA fires right
                                        # after its copy — shorter drain
                                        ow = blk(f"ow{tb}{et}{sub}_{rep}", w=w)
                                        nc.scalar.activation(ow[:], pw[:], Copy)
                                        nc.sync.dma_start(
                                            outT[et * P : (et + 1) * P, cols],
                                            ow[:])
                                        return
                                    # stage et pairs and write both with one
                                    # DMA: halves the output DMA count
                                    pr = et // 2
                                    if pr not in stages:
                                        stages[pr] = gp.tile(
                                            [P, 2, w], BF16, tag="ow2", bufs=3,
                                            name=f"ow{tb}{pr}{sub}_{rep}")
                                    st = stages[pr]
                                    nc.scalar.activation(st[:, et % 2, :], pw[:],
                                                         Copy)
                                    if et % 2 == 1:
                                        nc.sync.dma_start(
                                            outT.rearrange("(e p) t -> p e t", p=P)[
                                                :, 2 * pr : 2 * pr + 2, cols],
                                            stages.pop(pr)[:])

                                # zigzag: each et's oT1 half trails the next
                                # et's oT0 half, giving the last normalize
                                # chain extra runway
                                first_half(0)
                                for et in range(1, NE):
                                    first_half(et)
                                    second_half(et - 1)
                                second_half(NE - 1)
                        return emit

                    # h0 runs two ib-pairs ahead of h1 (ib0/ib1 were emitted
                    # inside the projection phase)
                    for ib in range(NIB):
                        if ib + 2 < NIB:
                            emit_attn(0, ib + 2)
                        if ib in (2, 4):
                            pending.append(wo_block((ib - 2) // 2))
                        emit_attn(1, ib)
                        if ib == 6:
                            pending.append(wo_block(2))
                    drain()
                    flush()
                    # last wo block in 256-col halves: shorter serial tail
                    wo_block(NTB - 1, w=256)()

    nc.compile()
    return nc


_NC = None


def _get_nc():
    global _NC
    if _NC is None:
        _NC = _build_nc()
    return _NC


def _prep_inputs(x, wq, wk, wv, wo):
    """Host-side shard prep. Returns in_maps list of 8 dicts (core = b*4+g)."""
    x = np.asarray(x, dtype=np.float32)
    wq = np.asarray(wq, dtype=np.float32)
    wk = np.asarray(wk, dtype=np.float32)
    wv = np.asarray(wv, dtype=np.float32)
    wo = np.asarray(wo, dtype=np.float32)
    bf16 = ml_dtypes.bfloat16

    # rope permutation of head-dim rows: per head, new order =
    # [pair-block 0 x1 | pair-block 0 x2 | pair-block 1 x1 | pair-block 1 x2]
    perm = np.empty(D, dtype=np.int64)
    for dp in range(2):
        base = dp * 256
        pairs = dp * 128 + np.arange(128)
        perm[base : base + 128] = 2 * pairs
        perm[base + 128 : base + 256] = 2 * pairs + 1
    full_perm = np.concatenate([h * D + perm for h in range(H)])

    scale = 1.0 / np.sqrt(np.float32(D))
    wq_p = (wq[full_perm] * scale).astype(np.float32)
    wk_p = wk[full_perm].astype(np.float32)

    # rope tables [D/2, T] fp16 (pair-index major)
    inv_freq = 1.0 / (10000.0 ** (np.arange(0, D, 2, dtype=np.float64) / D))
    ang = inv_freq[:, None] * np.arange(T, dtype=np.float64)[None, :]
    cosT = np.cos(ang).astype(np.float16)
    sinT = np.sin(ang).astype(np.float16)

    # additive causal masks for the 2 diagonal 128x256 sub-blocks
    rj = np.arange(P)[:, None]
    c = np.arange(IBW)[None, :]
    masks = np.empty((2 * P, IBW), dtype=np.float16)
    for q in range(2):
        masks[q * P : (q + 1) * P] = np.where(c >= 128 * q + rj, 0.0, MASK_NEG)


    in_maps = []
    for core in range(8):
        b, g = divmod(core, G)
        rows = slice(g * HPC * D, (g + 1) * HPC * D)
        in_maps.append({
            "xT": np.ascontiguousarray(x[b].T).astype(bf16),
            "wqT": np.ascontiguousarray(wq_p[rows].T).astype(bf16),
            "wkT": np.ascontiguousarray(wk_p[rows].T).astype(bf16),
            "wvT": np.ascontiguousarray(wv[rows].T).astype(bf16),
            "woT": np.ascontiguousarray(wo[:, rows].T).astype(bf16),
            "cosT": cosT,
            "sinT": sinT,
            "masks": masks,
        })
    return in_maps


def _assemble(results):
    """Sum the 4 TP partials per batch and transpose back to [B, T, E]."""
    out = np.empty((B, T, E), dtype=np.float32)
    for b in range(B):
        acc = results[b * G]["outT"].astype(np.float32)
        for g in range(1, G):
            acc = acc + results[b * G + g]["outT"].astype(np.float32)
        out[b] = acc.T
    return out


def kernel(x, wq, wk, wv, wo):
    nc = _get_nc()
    in_maps = _prep_inputs(x, wq, wk, wv, wo)
    res = run_bass_kernel_spmd(nc, in_maps, list(range(8)))
    return _assemble(res.results)
